# revision 44
# baseline (speedup 1.0000x reference)
"""Trainium2 Bass kernel for the DifferentiableMemory scatter_memory problem.

Data-parallel over 8 NeuronCores: batch B=32768 is sharded into 8 x 4096 rows.
Host side does layout only (transpose/cast/concat/weight repack); all NN math
(encoder MLP, cosine sims, top-k, importance net) runs on device.

v2: the K=768 cue contractions (encoder layer 1 + importance layer 1) run in
fp8e4m3 with DoubleRow perf mode (K=256 per matmul, 2 fp8 weights per PE
cell), halving both the PE time and the cue DMA bytes of the dominant sweep.
Everything downstream (h1, encoder L2, cosine sims, top-k) stays bf16 with
fp32 PSUM accumulation; measured end-to-end l2 rel err ~1.4e-2 (< 2e-2 gate).

Device dataflow (per core, 8 superblocks of 512 batch columns):
    xt        [128, 3, 2, 512] fp8   cue.T chunk-pairs (k = c*256 + j*128 + p)
    h1T       = gelu(W1.T @ xT + b1)          -> [256, 512] bf16   (3 DR MMs/half)
    ps_imp   += iw1_cue.T @ xT (3 DR MMs) + iw1_tail.T @ tail (bf16)
    encT      = W2.T @ h1T (bf16) ; encb = Identity(ps+b2), enc2 = Square(ps+b2)
                both on the ACT engine (every ACT table has identity+square,
                so the Gelu table stays resident; no table switches at all)
    ssq[b]    = ones.T @ enc2                 -> per-batch ||enc||^2 via PE
    rinv      = rsqrt(ssq) quake+2NR entirely on GpSimd (seeds on DVE)
    sims[b,n] = encb.T @ centT_scaled         -> [128, 500] fp32 (centT
                pre-divided by ||c||; divide by ||enc|| AFTER top-8: positive
                per-row scale preserves order; eps clamp never binds)
    top8      = nc.vector.max (one DVE instruction, sorted desc) -> take 5
    head      = tanh((himp @ iw2/2) + ib2/2) on ACT (tanh lives in the gelu
                table; sigmoid(z) = (tanh(z/2)+1)/2 folded into the epilogue:
                col5 = (tanh+1) * sum(emo)/8 with emo pre-divided by 8)
"""

import numpy as np
import ml_dtypes

BF16 = ml_dtypes.bfloat16
FP8 = ml_dtypes.float8_e4m3

N_CORES = 8
B = 32768
BL = B // N_CORES          # 4096 rows per core
SB = 512                   # superblock: batch columns per iteration
NSB = BL // SB             # 8 superblocks
Q = SB // 128              # 4 x 128-row tiles per superblock
D = 768
H1 = 256
E = 128
N = 500
K = 5
TOT = 902
NWARM = 7                  # dummy matmuls to cover the DMA ramp + HAM warmup

_CACHE = {}


def _build_nc(has_ist):
    """Build the device kernel. has_ist: include the internal_state chunk
    (False when it is all-zeros, making its contribution exactly zero)."""
    import concourse.bacc as bacc
    import concourse.bass as bass
    import concourse.tile as tile
    from concourse import mybir

    f32 = mybir.dt.float32
    bf16 = mybir.dt.bfloat16
    fp8 = mybir.dt.float8e4
    i32 = mybir.dt.int32
    AF = mybir.ActivationFunctionType
    AO = mybir.AluOpType
    DR = mybir.MatmulPerfMode.DoubleRow
    ts = bass.ts

    nc = bacc.Bacc(None, target_bir_lowering=False,
               enable_asserts=False, enable_partition_id=False)

    cueQ = nc.dram_tensor("cueQ", [128, NSB, 3, 2, SB], fp8, kind="ExternalInput")
    tailT = nc.dram_tensor("tailT", [6, BL], bf16, kind="ExternalInput")
    if has_ist:
        istT = nc.dram_tensor("istT", [E, BL], bf16, kind="ExternalInput")
    emo = nc.dram_tensor("emo", [128, BL // 128, 4], f32, kind="ExternalInput")
    w1 = nc.dram_tensor("w1", [128, 3, 2, H1 + 64], fp8, kind="ExternalInput")
    w2 = nc.dram_tensor("w2", [128, 2, E], bf16, kind="ExternalInput")
    iw1 = nc.dram_tensor("iw1", [128, 2 if has_ist else 1, 64], bf16,
                         kind="ExternalInput")
    iw2 = nc.dram_tensor("iw2", [64, 1], bf16, kind="ExternalInput")
    b1 = nc.dram_tensor("b1", [128, 2], f32, kind="ExternalInput")
    b2 = nc.dram_tensor("b2", [128, 1], f32, kind="ExternalInput")
    ib1 = nc.dram_tensor("ib1", [64, 1], f32, kind="ExternalInput")
    ib2 = nc.dram_tensor("ib2", [128, 1], f32, kind="ExternalInput")
    centT = nc.dram_tensor("centT", [128, N], bf16, kind="ExternalInput")
    out = nc.dram_tensor("out", [128, (BL // 128) * (K + 1)], f32,
                         kind="ExternalOutput")

    with tile.TileContext(nc) as tc:
        with (
            tc.tile_pool(name="const", bufs=1) as cpool,
            tc.tile_pool(name="work", bufs=3) as wpool,
            tc.tile_pool(name="acc", bufs=1) as apool,
            tc.tile_pool(name="small", bufs=2) as opool,
            tc.tile_pool(name="psA", bufs=3, space="PSUM") as psA,
            tc.tile_pool(name="psS", bufs=4, space="PSUM") as psS,
            tc.tile_pool(name="psT", bufs=1, space="PSUM") as psT,
        ):
            # PE warm-up burst: dummy matmuls gated only by a gpsimd memset,
            # covering the input-DMA ramp while accumulating HAM activity.
            scr = cpool.tile([128, SB], bf16)
            nc.gpsimd.memset(scr[:], 0.0)
            # quake constants early on gpsimd (cheap memsets, before the
            # gpsimd DMA issue slices) so sb0's rsqrt is never gated on them
            kmag = cpool.tile([128, Q], i32)
            nc.gpsimd.memset(kmag[:], 0x5F3759DF)
            c15 = cpool.tile([128, Q], f32)
            nc.gpsimd.memset(c15[:], 1.5)
            chalf = cpool.tile([128, Q], f32)
            nc.gpsimd.memset(chalf[:], 0.5)
            ps_warm = psS.tile([128, SB], f32, tag="sims")
            for _ in range(NWARM):
                nc.tensor.matmul(ps_warm[:], lhsT=scr[:, 0:128], rhs=scr[:],
                                 start=True, stop=True)
            # dummy 1-element activation: hoists the compiler-inserted
            # ACT_TABLE_LOADs (2x 1.28us) into the idle DMA-ramp window —
            # otherwise the second load lands right before sb0's first Gelu
            # and delays the whole ACT pipeline
            scrg = cpool.tile([128, 1], f32)
            nc.scalar.activation(scrg[:], scr[:, 0:1], AF.Gelu)

            # ---- consts. The head is DMA-ramp-bound, so the first loads
            # issue in parallel from three queues: w1 chunk 0 on gpsimd,
            # sb0's x chunks + remaining weights on sync, small biases on
            # early (pre-Gelu) ACT issue slices. sb0's first DoubleRow
            # matmul is gated on chunk 0 only (~210KB, not ~640KB).
            w1t = cpool.tile([128, 3, 2, H1 + 64], fp8)
            nc.gpsimd.dma_start(w1t[:, 0], w1[:, 0])

            onesE = cpool.tile([128, 1], bf16)
            nc.vector.memset(onesE[:], 1.0)

            b1t = cpool.tile([128, 2], f32)
            nc.gpsimd.dma_start(b1t[:], b1[:])
            # ACT is idle until the first Gelu (~2.7us) — two early issue
            # slices there are free and keep the gpsimd queue short
            b2t = cpool.tile([128, 1], f32)
            nc.scalar.dma_start(b2t[:], b2[:])
            ib2t = cpool.tile([128, 1], f32)
            nc.scalar.dma_start(ib2t[:], ib2[:])

            w2t = cpool.tile([128, 2, E], bf16)
            iw1t = cpool.tile([128, 2 if has_ist else 1, 64], bf16)
            iw2t = cpool.tile([64, 1], bf16)
            ib1t = cpool.tile([64, 1], f32)
            centTt = cpool.tile([128, N], bf16)
            emot = cpool.tile([128, BL // 128, 4], f32)
            nc.gpsimd.dma_start(ib1t[:], ib1[:])

            # accumulators; output assembly deferred off the main loop so
            # ACT keeps the Gelu table resident throughout (identity/square/
            # tanh/copy all live in the same table).
            XT = NSB * Q  # 32 tiles of 128 rows
            sg_all = apool.tile([128, XT], f32)
            esum8 = apool.tile([128, XT], f32)
            top8_all = apool.tile([128, XT, 8], f32)
            rinv_all = apool.tile([128, XT], f32)

            def front(sb):
                """Input DMA + the fp8 DoubleRow layer-1 sweeps + gelus.
                Emitted one superblock AHEAD of back(sb-1) so the PE never
                stalls on the ACT pipeline at superblock boundaries."""
                xt = wpool.tile([128, 3, 2, SB], fp8, tag="xt",
                                name=f"xt{sb}")
                if sb == 0:
                    nc.sync.dma_start(xt[:, 0], cueQ[:, 0, 0])
                    nc.sync.dma_start(w1t[:, 1], w1[:, 1])
                    nc.sync.dma_start(xt[:, 1], cueQ[:, 0, 1])
                    nc.sync.dma_start(w1t[:, 2], w1[:, 2])
                    nc.sync.dma_start(xt[:, 2], cueQ[:, 0, 2])
                else:
                    nc.sync.dma_start(xt[:], cueQ[:, sb])
                xtail = wpool.tile([6, SB], bf16, tag="xtail",
                                   name=f"xtail{sb}")
                nc.sync.dma_start(xtail[:], tailT[:, ts(sb, SB)])
                if sb == 0:
                    nc.sync.dma_start(iw1t[:], iw1[:])
                xti = None
                if has_ist:
                    xti = wpool.tile([128, SB], bf16, tag="xti",
                                     name=f"xti{sb}")
                    nc.sync.dma_start(xti[:], istT[:, ts(sb, SB)])

                h1 = wpool.tile([128, 2, SB], bf16, tag="h1", name=f"h1_{sb}")
                for half in range(2):
                    ps = psA.tile([128, SB], f32, tag="mm", name=f"psh{sb}")
                    for c in range(3):
                        nc.tensor.matmul(
                            ps[:],
                            lhsT=w1t[:, c, :, ts(half, 128)],
                            rhs=xt[:, c, :, :],
                            start=(c == 0),
                            stop=(c == 2),
                            perf_mode=DR,
                        )
                    nc.scalar.activation(
                        h1[:, half, :], ps[:], AF.Gelu,
                        bias=b1t[:, half : half + 1]
                    )
                ps_imp = psA.tile([64, SB], f32, tag="mm", name=f"psi{sb}")
                for c in range(3):
                    nc.tensor.matmul(
                        ps_imp[:], lhsT=w1t[:, c, :, 256 : 256 + 64],
                        rhs=xt[:, c, :, :],
                        start=(c == 0), stop=False, perf_mode=DR,
                    )
                if has_ist:
                    nc.tensor.matmul(
                        ps_imp[:], lhsT=iw1t[:, 1, :], rhs=xti[:],
                        start=False, stop=False,
                    )
                nc.tensor.matmul(
                    ps_imp[:], lhsT=iw1t[0:6, 0, :], rhs=xtail[:],
                    start=False, stop=True,
                )
                return h1, ps_imp

            def back(sb, h1, ps_imp):
                last = sb == NSB - 1
                if sb == 0:
                    # late consts: issued after sb1's x so the steady-state
                    # input stream is never starved
                    nc.sync.dma_start(w2t[:], w2[:])
                    nc.sync.dma_start(centTt[:], centT[:])
                    nc.gpsimd.dma_start(iw2t[:], iw2[:])
                    nc.gpsimd.dma_start(emot[:], emo[:])

                # ---- encoder layer 2 (bf16): encT = W2.T @ h1T ----
                ps_enc = psA.tile([128, SB], f32, tag="mm", name=f"pse{sb}")
                for c in range(2):
                    nc.tensor.matmul(
                        ps_enc[:],
                        lhsT=w2t[:, c, :],
                        rhs=h1[:, c, :],
                        start=(c == 0),
                        stop=(c == 1),
                    )

                # encb for sims, enc2 for ||enc||^2 — both on ACT from PSUM.
                # On the LAST superblock encb is quarter-split so the first
                # sims matmul (and its MAX8) starts ~0.4us after L2 instead
                # of waiting for the full-width op, with enc2 slotted between
                # quarters to keep the rsqrt path early too.
                encb = wpool.tile([128, SB], bf16, tag="encb",
                                  name=f"encb{sb}")
                enc2 = wpool.tile([128, SB], bf16, tag="enc2",
                                  name=f"enc2{sb}")
                if last:
                    for q in (0, 1):
                        nc.scalar.activation(encb[:, ts(q, 128)],
                                             ps_enc[:, ts(q, 128)],
                                             AF.Identity, bias=b2t[:])
                    nc.scalar.activation(enc2[:], ps_enc[:], AF.Square,
                                         bias=b2t[:])
                    for q in (2, 3):
                        nc.scalar.activation(encb[:, ts(q, 128)],
                                             ps_enc[:, ts(q, 128)],
                                             AF.Identity, bias=b2t[:])
                else:
                    nc.scalar.activation(encb[:], ps_enc[:], AF.Identity,
                                         bias=b2t[:])
                    nc.scalar.activation(enc2[:], ps_enc[:], AF.Square,
                                         bias=b2t[:])

                himp = wpool.tile([64, SB], bf16, tag="himp",
                                  name=f"himp{sb}")
                nc.scalar.activation(himp[:], ps_imp[:], AF.Gelu, bias=ib1t[:])

                if last:
                    # early epilogue: columns of superblocks 0..6 are final;
                    # assemble + ship them while sb7 still computes (the DVE
                    # ops wait on sb6's rinv/tanh and run during sb7's L1).
                    X0 = (NSB - 1) * Q  # 28
                    ot_a = opool.tile([128, X0, K + 1], f32, tag="ot_a")
                    nc.vector.tensor_mul(
                        ot_a[:, :, 0:K], top8_all[:, 0:X0, 0:K],
                        rinv_all[:, 0:X0].broadcast_to([128, X0, K]))
                    nc.vector.scalar_tensor_tensor(
                        ot_a[:, :, K], in0=sg_all[:, 0:X0], scalar=1.0,
                        in1=esum8[:, 0:X0], op0=AO.add, op1=AO.mult)
                    nc.sync.dma_start(out[:, 0 : X0 * (K + 1)], ot_a[:])

                def emit_sims(q):
                    ps_sims = psS.tile([128, N], f32, tag="sims",
                                       name=f"ps_sims{sb}_{q}")
                    nc.tensor.matmul(
                        ps_sims[:],
                        lhsT=encb[:, ts(q, 128)],
                        rhs=centTt[:],
                        start=True,
                        stop=True,
                    )
                    nc.vector.max(top8_all[:, sb * Q + q, :], ps_sims[:])

                def emit_ssq():
                    ps_ssq = psT.tile([128, Q], f32, tag="tiny",
                                      name=f"ps_ssq{sb}")
                    for q in range(Q):
                        nc.tensor.matmul(
                            ps_ssq[:, q : q + 1],
                            lhsT=enc2[:, ts(q, 128)],
                            rhs=onesE[:],
                            start=True,
                            stop=True,
                        )
                    return ps_ssq

                def emit_seeds(ps_ssq):
                    # quake seed + 0.5*x read straight from PSUM on DVE
                    y0i = opool.tile([128, Q], i32, tag="y0i")
                    nc.vector.tensor_single_scalar(
                        y0i[:], ps_ssq[:].bitcast(i32), 1,
                        AO.logical_shift_right)
                    nc.vector.tensor_tensor(y0i[:], kmag[:], y0i[:],
                                            AO.subtract)
                    hx = opool.tile([128, Q], f32, tag="hx")
                    nc.vector.tensor_mul(hx[:], ps_ssq[:], chalf[:])
                    return y0i, hx

                def emit_chain(y0i, hx):
                    # single Newton step on GpSimd: y <- y*(1.5-0.5*x*y^2);
                    # quake + 1 NR is ~0.17% max rel err on rinv, invisible
                    # next to the fp8 L1 noise. tensor_tensor only (the only
                    # ALU form GpSimd codegen accepts).
                    eng = nc.gpsimd
                    rs_t = opool.tile([128, Q], f32, tag="rs_t")
                    rs_u = opool.tile([128, Q], f32, tag="rs_u")
                    cur = y0i[:].bitcast(f32)
                    eng.tensor_mul(rs_t[:], cur, cur)
                    eng.tensor_mul(rs_u[:], rs_t[:], hx[:])
                    eng.tensor_tensor(rs_u[:], c15[:], rs_u[:], AO.subtract)
                    eng.tensor_mul(rinv_all[:, ts(sb, Q)], rs_u[:], cur)

                def emit_ic_tanh():
                    # importance head: tanh((himp @ iw2/2) + ib2/2) from PSUM
                    ps_ic = psT.tile([128, Q], f32, tag="tiny",
                                     name=f"ps_ic{sb}")
                    for q in range(Q):
                        nc.tensor.matmul(
                            ps_ic[:, q : q + 1],
                            lhsT=himp[:, ts(q, 128)],
                            rhs=iw2t[:],
                            start=True,
                            stop=True,
                        )
                    nc.scalar.activation(sg_all[:, ts(sb, Q)], ps_ic[:],
                                         AF.Tanh, bias=ib2t[:])

                if not last:
                    ps_ssq = emit_ssq()
                    y0i, hx = emit_seeds(ps_ssq)
                    emit_chain(y0i, hx)
                    for q in range(Q):
                        emit_sims(q)
                    emit_ic_tanh()
                else:
                    # tail order: sims/max8 lead; the rsqrt seeds slot into
                    # the DVE FIFO between max8s so the Newton chain finishes
                    # before the max8 drain does
                    emit_sims(0)
                    emit_sims(1)
                    ps_ssq = emit_ssq()
                    y0i, hx = emit_seeds(ps_ssq)
                    emit_sims(2)
                    emit_sims(3)
                    emit_chain(y0i, hx)
                    emit_ic_tanh()

                if sb == 0:
                    nc.vector.reduce_sum(
                        esum8[:], emot[:], axis=mybir.AxisListType.X
                    )

            # sequential drive: front(sb) then back(sb). (A 1-superblock
            # software pipeline — front(sb+1) before back(sb) — deadlocks
            # on hardware via an ACT-FIFO/PSUM-ring cycle; don't.)
            for sb_ in range(NSB):
                h1p, psimpp = front(sb_)
                back(sb_, h1p, psimpp)

            # ---- final epilogue: only the last superblock's 4 column
            # groups remain ----
            X0 = (NSB - 1) * Q
            ot_b = opool.tile([128, Q, K + 1], f32, tag="ot_b")
            nc.vector.tensor_mul(
                ot_b[:, :, 0:K], top8_all[:, X0:XT, 0:K],
                rinv_all[:, X0:XT].broadcast_to([128, Q, K]))
            nc.vector.scalar_tensor_tensor(
                ot_b[:, :, K], in0=sg_all[:, X0:XT], scalar=1.0,
                in1=esum8[:, X0:XT], op0=AO.add, op1=AO.mult)
            nc.sync.dma_start(out[:, X0 * (K + 1) :], ot_b[:])

    nc.compile()
    return nc


def _prep_inputs(has_ist, cue, internal_state, reward, timestamp,
                 emotional_state, centroids, enc_w1, enc_b1, enc_w2, enc_b2,
                 imp_w1, imp_b1, imp_w2, imp_b2):
    f32 = np.float32

    tail = np.empty((6, B), dtype=f32)
    tail[0] = reward[:, 0]
    tail[1] = timestamp[:, 0]
    tail[2:6] = emotional_state.T
    tail_bf = tail.astype(BF16)
    cue_q = cue.astype(FP8)
    ist_bf = internal_state.astype(BF16) if has_ist else None

    # fused L1 weights [768, 320] -> fp8 DoubleRow layout [128, 3, 2, 320]
    # with contraction row k = c*256 + j*128 + p
    w1e = np.concatenate([enc_w1, imp_w1[:D]], axis=1)       # [768, 320]
    w1 = np.ascontiguousarray(
        w1e.astype(FP8).reshape(3, 2, 128, H1 + 64).transpose(2, 0, 1, 3)
    )
    w2 = np.ascontiguousarray(
        enc_w2.astype(BF16).reshape(2, 128, E).transpose(1, 0, 2)
    )
    nchi = 2 if has_ist else 1
    iw1p = np.zeros((nchi * 128, 64), dtype=f32)
    iw1p[0:6] = imp_w1[TOT - 6 : TOT]            # chunk 0 = reward/ts/emo tail
    if has_ist:
        iw1p[128 : 128 + E] = imp_w1[D : D + E]  # chunk 1 = internal_state
    iw1 = np.ascontiguousarray(
        iw1p.astype(BF16).reshape(nchi, 128, 64).transpose(1, 0, 2)
    )
    # halved head weights: sigmoid(z) = (tanh(z/2)+1)/2, tanh evaluated on ACT
    iw2 = np.ascontiguousarray((imp_w2 * 0.5).astype(BF16).reshape(64, 1))
    b1 = np.ascontiguousarray(enc_b1.astype(f32).reshape(2, 128).T)
    b2 = np.ascontiguousarray(enc_b2.astype(f32).reshape(128, 1))
    ib1 = np.ascontiguousarray(imp_b1.astype(f32).reshape(64, 1))
    ib2 = np.full((128, 1), float(np.asarray(imp_b2).reshape(-1)[0]) * 0.5,
                  dtype=f32)

    cn = np.linalg.norm(centroids.astype(f32), axis=1)
    centT = np.ascontiguousarray((centroids / cn[:, None]).T).astype(BF16)

    shared = dict(w1=w1, w2=w2, iw1=iw1, iw2=iw2, b1=b1, b2=b2, ib1=ib1,
                  ib2=ib2, centT=centT)
    in_maps = []
    for i in range(N_CORES):
        sl = slice(i * BL, (i + 1) * BL)
        m = dict(shared)
        # cueQ[p, sb, c, j, n] = cue.T[c*256 + j*128 + p, sb*SB + n]
        # (per-superblock slices contiguous per partition: one 3KB DMA row)
        m["cueQ"] = np.ascontiguousarray(
            cue_q[sl].T.reshape(3, 2, 128, NSB, SB).transpose(2, 3, 0, 1, 4)
        )
        m["tailT"] = np.ascontiguousarray(tail_bf[:, sl])
        if has_ist:
            m["istT"] = np.ascontiguousarray(ist_bf[sl].T)
        # device-friendly emo layout, pre-divided by 8 so the epilogue is
        # col5 = (tanh+1) * esum8:  emo_dev[p, x, e] = emotional[x*128+p, e]/8
        m["emo"] = np.ascontiguousarray(
            (emotional_state[sl].astype(f32) * 0.125)
            .reshape(BL // 128, 128, 4).transpose(1, 0, 2)
        )
        in_maps.append(m)
    return in_maps


def kernel(cue, internal_state, reward, timestamp, emotional_state, centroids,
           enc_w1, enc_b1, enc_w2, enc_b2, imp_w1, imp_b1, imp_w2, imp_b2,
           top_k, **run_kwargs):
    assert int(top_k) == K, f"kernel hardcodes top_k={K}, got {top_k}"
    from concourse.bass_utils import run_bass_kernel_spmd

    has_ist = bool(np.any(internal_state))
    if ("nc", has_ist) not in _CACHE:
        _CACHE[("nc", has_ist)] = _build_nc(has_ist)
    nc = _CACHE[("nc", has_ist)]

    in_maps = _prep_inputs(
        has_ist,
        np.asarray(cue, np.float32), np.asarray(internal_state, np.float32),
        np.asarray(reward, np.float32), np.asarray(timestamp, np.float32),
        np.asarray(emotional_state, np.float32),
        np.asarray(centroids, np.float32),
        np.asarray(enc_w1, np.float32), np.asarray(enc_b1, np.float32),
        np.asarray(enc_w2, np.float32), np.asarray(enc_b2, np.float32),
        np.asarray(imp_w1, np.float32), np.asarray(imp_b1, np.float32),
        np.asarray(imp_w2, np.float32), np.asarray(imp_b2, np.float32),
    )
    res = run_bass_kernel_spmd(
        nc, in_maps, core_ids=list(range(N_CORES)), **run_kwargs
    )
    # device out is [128, XT*6] with out_dev[p, x*6+j] = out[x*128+p, j]
    parts = []
    for i in range(N_CORES):
        od = res.results[i]["out"].reshape(128, BL // 128, K + 1)
        parts.append(np.ascontiguousarray(od.transpose(1, 0, 2)).reshape(BL, K + 1))
    out = np.concatenate(parts, axis=0)
    _CACHE["last_results"] = res
    return out
